# revision 3
# baseline (speedup 1.0000x reference)
import sys
for p in ('/opt/trn_rl_repo', '/opt/pypackages'):
    if p not in sys.path:
        sys.path.insert(0, p)
import numpy as np
from concourse import bass, bacc, tile, mybir
from concourse import bass_utils

B, C, T, K = 4, 64, 4096, 4
NCORES = 8
OS = T // NCORES          # 512 per-core token slice
BC = B * C                # 256
NKT = T // 128            # 32 contraction tiles
f32 = mybir.dt.float32
f16 = mybir.dt.float16
u32 = mybir.dt.uint32

_cache = {}


def _build_l1():
    """Phase 1, core j: q/k (fp16 split-3) and v (fp16) projections for the
    o-slice [j*512,(j+1)*512); emit qn hi/lo, k hi/lo (fp16) and the conv-folded
    value tables u[p, b, k, m, c] = (conv_w_k @ v_b)[c, m*128+p]."""
    nc = bacc.Bacc("TRN2", target_bir_lowering=False, debug=False, num_devices=NCORES)
    XH = nc.dram_tensor("xh", [128, NKT, BC], f16, kind="ExternalInput").ap()
    XL = nc.dram_tensor("xl", [128, NKT, BC], f16, kind="ExternalInput").ap()
    WQH = nc.dram_tensor("wqh", [128, NKT, OS], f16, kind="ExternalInput").ap()
    WQL = nc.dram_tensor("wql", [128, NKT, OS], f16, kind="ExternalInput").ap()
    WKH = nc.dram_tensor("wkh", [128, NKT, OS], f16, kind="ExternalInput").ap()
    WKL = nc.dram_tensor("wkl", [128, NKT, OS], f16, kind="ExternalInput").ap()
    WV = nc.dram_tensor("wv", [128, NKT, OS], f16, kind="ExternalInput").ap()
    CW = nc.dram_tensor("cw", [2 * C, K * C], f16, kind="ExternalInput").ap()
    QNH = nc.dram_tensor("qnh_o", [2, 128, OS], f16, kind="ExternalOutput").ap()
    QNL = nc.dram_tensor("qnl_o", [2, 128, OS], f16, kind="ExternalOutput").ap()
    KH = nc.dram_tensor("kh_o", [2, 128, OS], f16, kind="ExternalOutput").ap()
    KL = nc.dram_tensor("kl_o", [2, 128, OS], f16, kind="ExternalOutput").ap()
    UO = nc.dram_tensor("u_o", [128, B, K, 4, C], f16, kind="ExternalOutput").ap()

    H = NKT // 4  # 8-kt chunks (1 MB per fp16 plane) for fine-grained streaming
    NHF = NKT // H

    with tile.TileContext(nc) as tc:
        with tc.tile_pool(name="xp", bufs=1) as xp, \
             tc.tile_pool(name="wp", bufs=6) as wp, \
             tc.tile_pool(name="sp", bufs=2) as sp, \
             tc.tile_pool(name="cp", bufs=1) as cp, \
             tc.tile_pool(name="up", bufs=1) as up, \
             tc.tile_pool(name="pp", bufs=2, space="PSUM") as pp, \
             tc.tile_pool(name="pu", bufs=1, space="PSUM") as pu:
            cw = cp.tile([2 * C, K * C], f16, tag="cw")
            nc.scalar.dma_start(out=cw[...], in_=CW[...])
            xh = xp.tile([128, NKT, BC], f16, tag="xh")
            xl = xp.tile([128, NKT, BC], f16, tag="xl")
            ones_r = cp.tile([128, 1], f32, tag="ones_r")
            nc.vector.memset(ones_r[:, :], 1.0)
            ones_b = cp.tile([1, C], f32, tag="ones_b")
            nc.vector.memset(ones_b[:, :], 1.0)

            # ---- V first (single fp16 matmul) so u tables overlap q/k ----
            vsb = {}
            vaccs = [pp.tile([128, OS], f32, tag=f"acc{mt}", name=f"vacc{mt}")
                     for mt in range(2)]
            for hf in range(NHF):
                nc.sync.dma_start(out=xh[:, hf * H:(hf + 1) * H, :],
                                  in_=XH[:, hf * H:(hf + 1) * H, :])
                nc.scalar.dma_start(out=xl[:, hf * H:(hf + 1) * H, :],
                                    in_=XL[:, hf * H:(hf + 1) * H, :])
                wv = wp.tile([128, H, OS], f16, tag="w")
                nc.sync.dma_start(out=wv[...], in_=WV[:, hf * H:(hf + 1) * H, :])
                for t in range(H):
                    kt = hf * H + t
                    for mt in range(2):
                        nc.tensor.matmul(out=vaccs[mt][:, :],
                                         lhsT=xh[:, kt, mt * 128:(mt + 1) * 128],
                                         rhs=wv[:, t, :],
                                         start=(kt == 0), stop=(kt == NKT - 1))
            for mt in range(2):
                v16 = sp.tile([128, OS], f16, tag=f"vsb{mt}")
                nc.scalar.copy(out=v16[:, :], in_=vaccs[mt][:, :])
                vsb[mt] = v16

            # ---- U tables: u[b,k] = v_b^T @ cw_k (overlaps q stream) ----
            uall = up.tile([128, B, K, 4, C], f16, tag="uall")
            for b in range(B):
                off = (b % 2) * C
                vt = vsb[b // 2][off:off + C, :]  # [64, 512] f16
                for k in range(K):
                    for m in range(4):
                        pt = pu.tile([128, C], f32, tag="pu", bufs=2)
                        nc.tensor.matmul(out=pt[:, :],
                                         lhsT=vt[:, m * 128:(m + 1) * 128],
                                         rhs=cw[off:off + C, k * C:(k + 1) * C],
                                         start=True, stop=True)
                        nc.scalar.copy(out=uall[:, b, k, m, :], in_=pt[:, :])
            nc.gpsimd.dma_start(out=UO[...], in_=uall[...])

            # ---- Q then K: split-3 fp16 matmuls ----
            qsb, ksb = {}, {}
            for which, WHt, WLt, store in (("q", WQH, WQL, qsb),
                                           ("k", WKH, WKL, ksb)):
                accs = [pp.tile([128, OS], f32, tag=f"acc{mt}",
                                name=f"acc_{which}{mt}") for mt in range(2)]
                for hf in range(NHF):
                    wh = wp.tile([128, H, OS], f16, tag="w")
                    wl = wp.tile([128, H, OS], f16, tag="w")
                    nc.sync.dma_start(out=wh[...], in_=WHt[:, hf * H:(hf + 1) * H, :])
                    nc.scalar.dma_start(out=wl[...], in_=WLt[:, hf * H:(hf + 1) * H, :])
                    last_hf = hf == NHF - 1
                    # mt-major on the last chunk: acc[mt] closes as soon as its
                    # own kt stream ends, so the fp16 split overlaps the rest
                    mt_t = ([(mt, t) for mt in range(2) for t in range(H)]
                            if last_hf else
                            [(mt, t) for t in range(H) for mt in range(2)])
                    for mt, t in mt_t:
                        kt = hf * H + t
                        first = (kt == 0)
                        last = (kt == NKT - 1)
                        lh = xh[:, kt, mt * 128:(mt + 1) * 128]
                        ll = xl[:, kt, mt * 128:(mt + 1) * 128]
                        nc.tensor.matmul(out=accs[mt][:, :], lhsT=lh,
                                         rhs=wh[:, t, :], start=first, stop=False)
                        nc.tensor.matmul(out=accs[mt][:, :], lhsT=lh,
                                         rhs=wl[:, t, :], start=False, stop=False)
                        nc.tensor.matmul(out=accs[mt][:, :], lhsT=ll,
                                         rhs=wh[:, t, :], start=False, stop=last)
                        if last_hf and t == H - 1:
                            res = sp.tile([128, OS], f32, tag=f"{which}sb{mt}",
                                          name=f"{which}sb{mt}")
                            nc.scalar.copy(out=res[:, :], in_=accs[mt][:, :])
                            store[mt] = res
                if which == "q":
                    # qn = q / ||q||_col + hi/lo split (overlaps k stream)
                    for mt in range(2):
                        qn = sp.tile([128, OS], f32, tag=f"qn{mt}", name=f"qn{mt}")
                        for half in range(2):
                            off = half * C
                            q_b = qsb[mt][off:off + C, :]
                            sq = sp.tile([128, OS], f32, tag="sq")
                            nc.scalar.square(out=sq[off:off + C, :], in_=q_b)
                            pn = pu.tile([1, OS], f32, tag="pn")
                            nc.tensor.matmul(out=pn[:, :], lhsT=ones_r[off:off + C, :],
                                             rhs=sq[off:off + C, :], start=True, stop=True)
                            nrm = sp.tile([1, OS], f32, tag="nrm")
                            nc.scalar.sqrt(out=nrm[:, :], in_=pn[:, :])
                            rcp = sp.tile([1, OS], f32, tag="rcp")
                            nc.vector.reciprocal(out=rcp[:, :], in_=nrm[:, :])
                            pb = pu.tile([128, OS], f32, tag="pb")
                            nc.tensor.matmul(out=pb[off:off + C, :], lhsT=ones_b[:, :],
                                             rhs=rcp[:, :], start=True, stop=True)
                            bc = sp.tile([128, OS], f32, tag="bc")
                            nc.scalar.copy(out=bc[off:off + C, :], in_=pb[off:off + C, :])
                            nc.vector.tensor_mul(out=qn[off:off + C, :], in0=q_b,
                                                 in1=bc[off:off + C, :])
                        qh16 = sp.tile([128, OS], f16, tag=f"qh16_{mt}", name=f"qh16_{mt}")
                        nc.scalar.copy(out=qh16[:, :], in_=qn[:, :])
                        qh32 = sp.tile([128, OS], f32, tag="qh32")
                        nc.scalar.copy(out=qh32[:, :], in_=qh16[:, :])
                        ql16 = sp.tile([128, OS], f16, tag=f"ql16_{mt}", name=f"ql16_{mt}")
                        nc.vector.tensor_sub(out=ql16[:, :], in0=qn[:, :], in1=qh32[:, :])
                        nc.sync.dma_start(out=QNH[mt], in_=qh16[:, :])
                        nc.sync.dma_start(out=QNL[mt], in_=ql16[:, :])

            # ---- k hi/lo split + store ----
            for mt in range(2):
                kh16 = sp.tile([128, OS], f16, tag=f"kh16_{mt}", name=f"kh16_{mt}")
                nc.scalar.copy(out=kh16[:, :], in_=ksb[mt][:, :])
                kh32 = sp.tile([128, OS], f32, tag="kh32")
                nc.scalar.copy(out=kh32[:, :], in_=kh16[:, :])
                kl16 = sp.tile([128, OS], f16, tag=f"kl16_{mt}", name=f"kl16_{mt}")
                nc.vector.tensor_sub(out=kl16[:, :], in0=ksb[mt][:, :], in1=kh32[:, :])
                nc.sync.dma_start(out=KH[mt], in_=kh16[:, :])
                nc.sync.dma_start(out=KL[mt], in_=kl16[:, :])
    nc.compile()
    return nc


def _build_l2():
    """Phase 2, core j: rows t in [j*512,(j+1)*512) for all batches.
    sim via packed split-3 (2 matmuls per 512-chunk), exact top-4 via
    max/max_index on fp32, gather-sum from combined u table, partial out."""
    nc = bacc.Bacc("TRN2", target_bir_lowering=False, debug=False, num_devices=NCORES)
    QHD = nc.dram_tensor("qhd", [B, 128, T], f16, kind="ExternalInput").ap()
    QLP = nc.dram_tensor("qlp", [2, 128, T], f16, kind="ExternalInput").ap()
    KJP = nc.dram_tensor("kjp", [B, 128, OS], f16, kind="ExternalInput").ap()
    WOT = nc.dram_tensor("wot", [128, 4, T], f16, kind="ExternalInput").ap()
    UT = [nc.dram_tensor(f"ut{b}", [K * T, C], f16, kind="ExternalInput").ap()
          for b in range(B)]
    OUT = nc.dram_tensor("out_o", [2, 128, T], f32, kind="ExternalOutput").ap()

    NCH = T // 512  # 8 s-chunks

    with tile.TileContext(nc) as tc:
        with tc.tile_pool(name="qp", bufs=1) as qp, \
             tc.tile_pool(name="wp", bufs=1) as wp, \
             tc.tile_pool(name="sp", bufs=3) as sp, \
             tc.tile_pool(name="simp", bufs=2) as simp, \
             tc.tile_pool(name="yp", bufs=1) as yp, \
             tc.tile_pool(name="pp", bufs=2, space="PSUM") as pp, \
             tc.tile_pool(name="po", bufs=2, space="PSUM") as po:
            kjp, qhd, qlp = {}, {}, {}
            kt0 = qp.tile([128, OS], f16, tag="kjp0", name="kjp0")
            nc.sync.dma_start(out=kt0[...], in_=KJP[0])
            kjp[0] = kt0
            # b=0 q tiles stream in quarters so block (0,0) starts early
            qt0 = qp.tile([128, T], f16, tag="qhd0", name="qhd0")
            ql0 = qp.tile([128, T], f16, tag="qlp0", name="qlp0")
            for c4 in range(4):
                nc.sync.dma_start(out=qt0[:, c4 * 1024:(c4 + 1) * 1024],
                                  in_=QHD[0][:, c4 * 1024:(c4 + 1) * 1024])
                nc.scalar.dma_start(out=ql0[:, c4 * 1024:(c4 + 1) * 1024],
                                    in_=QLP[0][:, c4 * 1024:(c4 + 1) * 1024])
            qhd[0] = qt0
            qlp[0] = ql0
            for b in range(1, B):
                kt_ = qp.tile([128, OS], f16, tag=f"kjp{b}", name=f"kjp{b}")
                nc.sync.dma_start(out=kt_[...], in_=KJP[b])
                kjp[b] = kt_
            for b in range(1, B):
                qt = qp.tile([128, T], f16, tag=f"qhd{b}", name=f"qhd{b}")
                nc.sync.dma_start(out=qt[...], in_=QHD[b])
                qhd[b] = qt
            qt = qp.tile([128, T], f16, tag="qlp1", name="qlp1")
            nc.scalar.dma_start(out=qt[...], in_=QLP[1])
            qlp[1] = qt
            wot = wp.tile([128, 4, T], f16, tag="wot")
            for kt in range(4):
                nc.sync.dma_start(out=wot[:, kt, :], in_=WOT[:, kt, :])

            ytp = {}
            for pair in range(2):
                for kt in range(4):
                    ytp[(pair, kt)] = yp.tile([128, 128], f16, tag=f"yt{pair}{kt}", name=f"ytp{pair}{kt}")

            def do_block(b, i):
                """sim+topk+gather for token block i (128 rows) of batch b."""
                loff = (b % 2) * C  # partition offset of kh within kjp[b]
                lhs_full = kjp[b][:, i * 128:(i + 1) * 128]
                lhs_h = kjp[b][loff:loff + C, i * 128:(i + 1) * 128]
                sim = simp.tile([128, T], f32, tag="sim", bufs=3)
                for ch2 in range(NCH // 2):
                    ps = pp.tile([128, 1024], f32, tag="ps")
                    for half in range(2):
                        ch = ch2 * 2 + half
                        po_s = ps[:, half * 512:(half + 1) * 512]
                        nc.tensor.matmul(out=po_s, lhsT=lhs_full,
                                         rhs=qhd[b][:, ch * 512:(ch + 1) * 512],
                                         start=True, stop=False)
                        nc.tensor.matmul(out=po_s, lhsT=lhs_h,
                                         rhs=qlp[b // 2][loff:loff + C,
                                                         ch * 512:(ch + 1) * 512],
                                         start=False, stop=True)
                    nc.scalar.copy(out=sim[:, ch2 * 1024:(ch2 + 1) * 1024],
                                   in_=ps[:, :])
                m8 = sp.tile([128, 8], f32, tag="m8")
                i8 = sp.tile([128, 8], u32, tag="i8")
                nc.vector.max(out=m8[:, :], in_=sim[:, :])
                nc.vector.max_index(out=i8[:, :], in_max=m8[:, :], in_values=sim[:, :])
                gth = sp.tile([128, K, C], f16, tag="gth")
                for k in range(K):
                    nc.gpsimd.indirect_dma_start(
                        out=gth[:, k, :], out_offset=None,
                        in_=UT[b][:, :],
                        in_offset=bass.IndirectOffsetOnAxis(ap=i8[:, k:k + 1], axis=0),
                        element_offset=k * T * C)
                t0 = sp.tile([128, C], f16, tag="t0")
                t1 = sp.tile([128, C], f16, tag="t1")
                nc.gpsimd.tensor_add(out=t0[:, :], in0=gth[:, 0, :], in1=gth[:, 1, :])
                nc.gpsimd.tensor_add(out=t1[:, :], in0=gth[:, 2, :], in1=gth[:, 3, :])
                dst = ytp[(b // 2, i)][:, (b % 2) * C:(b % 2) * C + C]
                nc.gpsimd.tensor_add(out=dst, in0=t0[:, :], in1=t1[:, :])

            out_ps = {}

            def out_prefill(pair):
                # accumulate kt=0..2 into 4 open psum groups (4 banks) before
                # the pair's last token block lands
                for ch2 in range(NCH // 2):
                    ps = po.tile([128, 512], f32, tag="po", bufs=4,
                                 name=f"po{pair}{ch2}")
                    out_ps[(pair, ch2)] = ps
                    for kt in range(3):
                        nc.tensor.matmul(out=ps[:, :], lhsT=ytp[(pair, kt)][:, :],
                                         rhs=wot[:, kt, ch2 * 512:(ch2 + 1) * 512],
                                         start=(kt == 0), stop=False)

            def do_out(pair):
                ob = simp.tile([128, T], f32, tag="ob")
                for ch2 in range(NCH // 2):
                    ps = out_ps[(pair, ch2)]
                    nc.tensor.matmul(out=ps[:, :], lhsT=ytp[(pair, 3)][:, :],
                                     rhs=wot[:, 3, ch2 * 512:(ch2 + 1) * 512],
                                     start=False, stop=True)
                    nc.scalar.copy(out=ob[:, ch2 * 512:(ch2 + 1) * 512], in_=ps[:, :])
                    eng = nc.sync if ch2 % 2 == 0 else nc.gpsimd
                    eng.dma_start(out=OUT[pair][:, ch2 * 512:(ch2 + 1) * 512],
                                  in_=ob[:, ch2 * 512:(ch2 + 1) * 512])
                for ch2 in range(NCH // 2, NCH):
                    ps = po.tile([128, 512], f32, tag="po", bufs=4,
                                 name=f"po{pair}{ch2}")
                    for kt in range(4):
                        nc.tensor.matmul(out=ps[:, :], lhsT=ytp[(pair, kt)][:, :],
                                         rhs=wot[:, kt, ch2 * 512:(ch2 + 1) * 512],
                                         start=(kt == 0), stop=(kt == 3))
                    nc.scalar.copy(out=ob[:, ch2 * 512:(ch2 + 1) * 512], in_=ps[:, :])
                    eng = nc.sync if ch2 % 2 == 0 else nc.gpsimd
                    eng.dma_start(out=OUT[pair][:, ch2 * 512:(ch2 + 1) * 512],
                                  in_=ob[:, ch2 * 512:(ch2 + 1) * 512])

            for b in range(2):
                for i in range(4):
                    if b == 1 and i == 3:
                        out_prefill(0)
                    do_block(b, i)
            do_out(0)
            for b in range(2, 4):
                for i in range(4):
                    if b == 3 and i == 3:
                        out_prefill(1)
                    do_block(b, i)
            do_out(1)
    nc.compile()
    return nc


def _split16(a):
    h = a.astype(np.float16)
    l = (a.astype(np.float32) - h.astype(np.float32)).astype(np.float16)
    return h, l


def _sw(a):
    # [T, W] -> [128, T//128, W] with [p, kt, w] = a[kt*128+p, w]
    return np.ascontiguousarray(a.reshape(T // 128, 128, -1).transpose(1, 0, 2))


def kernel(x, Wq, Wk, Wv, Wo, conv_w, conv_b):
    x = np.asarray(x, np.float32)
    Wq = np.asarray(Wq, np.float32); Wk = np.asarray(Wk, np.float32)
    Wv = np.asarray(Wv, np.float32); Wo = np.asarray(Wo, np.float32)
    conv_w = np.asarray(conv_w, np.float32); conv_b = np.asarray(conv_b, np.float32)

    if "l1" not in _cache:
        _cache["l1"] = _build_l1()
    if "l2" not in _cache:
        _cache["l2"] = _build_l2()

    xT = np.ascontiguousarray(x.transpose(2, 0, 1).reshape(T, BC))  # [t, b*64+c]
    xh, xl = _split16(xT)
    xh, xl = _sw(xh), _sw(xl)
    WqT, WkT = Wq.T, Wk.T
    WvT16 = np.ascontiguousarray(Wv.T).astype(np.float16)
    cw1 = np.ascontiguousarray(conv_w.transpose(1, 2, 0).reshape(C, K * C)).astype(np.float16)
    cw = np.concatenate([cw1, cw1], axis=0)

    in_maps = []
    for j in range(NCORES):
        sl = slice(j * OS, (j + 1) * OS)
        wqh, wql = _split16(np.ascontiguousarray(WqT[:, sl]))
        wkh, wkl = _split16(np.ascontiguousarray(WkT[:, sl]))
        in_maps.append({"xh": xh, "xl": xl,
                        "wqh": _sw(wqh), "wql": _sw(wql),
                        "wkh": _sw(wkh), "wkl": _sw(wkl),
                        "wv": _sw(np.ascontiguousarray(WvT16[:, sl])), "cw": cw})
    r1 = bass_utils.run_bass_kernel_spmd(_cache["l1"], in_maps, core_ids=list(range(NCORES)))

    # host: assemble full qn hi/lo [BC, T], k hi/lo, u tables
    qh_full = np.concatenate([r1.results[j]["qnh_o"].reshape(BC, OS)
                              for j in range(NCORES)], axis=1)  # [256, T]
    ql_full = np.concatenate([r1.results[j]["qnl_o"].reshape(BC, OS)
                              for j in range(NCORES)], axis=1)
    ut = {}
    for b in range(B):
        # u_o[p, b, k, m, c] -> table [K*T, C] rows k*T + (j*512 + m*128 + p)
        tabs = []
        for k in range(K):
            cols = [r1.results[j]["u_o"][:, b, k].transpose(1, 0, 2).reshape(OS, C)
                    for j in range(NCORES)]
            tabs.append(np.concatenate(cols, axis=0))  # [T, C]
        ut[b] = np.ascontiguousarray(np.concatenate(tabs, axis=0))  # [K*T, C]

    # per-batch duplicated qh and packed ql
    qhd = np.empty((B, 128, T), np.float16)
    qlp = np.empty((2, 128, T), np.float16)
    for b in range(B):
        qh_b = qh_full[b * C:(b + 1) * C]  # [64, T]
        qhd[b, :C] = qh_b
        qhd[b, C:] = qh_b
        qlp[b // 2, (b % 2) * C:(b % 2) * C + C] = ql_full[b * C:(b + 1) * C]

    wotT = np.ascontiguousarray(Wo.T).astype(np.float16)  # [T, T] rows t

    in_maps2 = []
    for j in range(NCORES):
        sl = slice(j * OS, (j + 1) * OS)
        khj = r1.results[j]["kh_o"].reshape(BC, OS)
        klj = r1.results[j]["kl_o"].reshape(BC, OS)
        kjp = np.empty((B, 128, OS), np.float16)
        for b in range(B):
            kh_b = khj[b * C:(b + 1) * C]
            kl_b = klj[b * C:(b + 1) * C]
            if b % 2 == 0:
                kjp[b, :C], kjp[b, C:] = kh_b, kl_b
            else:
                kjp[b, :C], kjp[b, C:] = kl_b, kh_b
        wotj = np.ascontiguousarray(
            wotT[sl, :].reshape(4, 128, T).transpose(1, 0, 2))
        m = {"qhd": qhd, "qlp": qlp, "kjp": kjp, "wot": wotj}
        for b in range(B):
            m[f"ut{b}"] = ut[b]
        in_maps2.append(m)
    r2 = bass_utils.run_bass_kernel_spmd(_cache["l2"], in_maps2, core_ids=list(range(NCORES)))

    out = np.zeros((B, C, T), np.float32)
    for j in range(NCORES):
        oo = r2.results[j]["out_o"]  # [2, 128, T]
        for b in range(B):
            out[b] += oo[b // 2, (b % 2) * C:(b % 2) * C + C, :]
    out += (conv_b[:, None] * Wo.sum(axis=1)[None, :])[None, :, :]
    return out


# revision 4
# speedup vs baseline: 1.0164x; 1.0164x over previous
import sys
for p in ('/opt/trn_rl_repo', '/opt/pypackages'):
    if p not in sys.path:
        sys.path.insert(0, p)
import numpy as np
from concourse import bass, bacc, tile, mybir
from concourse import bass_utils

B, C, T, K = 4, 64, 4096, 4
NCORES = 8
OS = T // NCORES          # 512 per-core token slice
BC = B * C                # 256
NKT = T // 128            # 32 contraction tiles
f32 = mybir.dt.float32
f16 = mybir.dt.float16
u32 = mybir.dt.uint32

_cache = {}


def _build_l1():
    """Phase 1, core j: q/k (fp16 split-3) and v (fp16) projections for the
    o-slice [j*512,(j+1)*512); emit qn hi/lo, k hi/lo (fp16) and the conv-folded
    value tables u[p, b, k, m, c] = (conv_w_k @ v_b)[c, m*128+p]."""
    nc = bacc.Bacc("TRN2", target_bir_lowering=False, debug=False, num_devices=NCORES)
    XH = nc.dram_tensor("xh", [128, NKT, BC], f16, kind="ExternalInput").ap()
    XL = nc.dram_tensor("xl", [128, NKT, BC], f16, kind="ExternalInput").ap()
    WQH = nc.dram_tensor("wqh", [128, NKT, OS], f16, kind="ExternalInput").ap()
    WQL = nc.dram_tensor("wql", [128, NKT, OS], f16, kind="ExternalInput").ap()
    WKH = nc.dram_tensor("wkh", [128, NKT, OS], f16, kind="ExternalInput").ap()
    WKL = nc.dram_tensor("wkl", [128, NKT, OS], f16, kind="ExternalInput").ap()
    WV = nc.dram_tensor("wv", [128, NKT, OS], f16, kind="ExternalInput").ap()
    CW = nc.dram_tensor("cw", [2 * C, K * C], f16, kind="ExternalInput").ap()
    QNH = nc.dram_tensor("qnh_o", [2, 128, OS], f16, kind="ExternalOutput").ap()
    QNL = nc.dram_tensor("qnl_o", [2, 128, OS], f16, kind="ExternalOutput").ap()
    KH = nc.dram_tensor("kh_o", [2, 128, OS], f16, kind="ExternalOutput").ap()
    KL = nc.dram_tensor("kl_o", [2, 128, OS], f16, kind="ExternalOutput").ap()
    UO = nc.dram_tensor("u_o", [128, B, K, 4, C], f16, kind="ExternalOutput").ap()

    H = NKT // 4  # 8-kt chunks (1 MB per fp16 plane) for fine-grained streaming
    NHF = NKT // H

    with tile.TileContext(nc) as tc:
        with tc.tile_pool(name="xp", bufs=1) as xp, \
             tc.tile_pool(name="wp", bufs=6) as wp, \
             tc.tile_pool(name="sp", bufs=2) as sp, \
             tc.tile_pool(name="cp", bufs=1) as cp, \
             tc.tile_pool(name="up", bufs=1) as up, \
             tc.tile_pool(name="pp", bufs=2, space="PSUM") as pp, \
             tc.tile_pool(name="pu", bufs=1, space="PSUM") as pu:
            cw = cp.tile([2 * C, K * C], f16, tag="cw")
            nc.scalar.dma_start(out=cw[...], in_=CW[...])
            xh = xp.tile([128, NKT, BC], f16, tag="xh")
            xl = xp.tile([128, NKT, BC], f16, tag="xl")
            ones_r = cp.tile([128, 1], f32, tag="ones_r")
            nc.vector.memset(ones_r[:, :], 1.0)
            ones_b = cp.tile([1, C], f32, tag="ones_b")
            nc.vector.memset(ones_b[:, :], 1.0)

            # ---- V first (single fp16 matmul) so u tables overlap q/k ----
            vsb = {}
            vaccs = [pp.tile([128, OS], f32, tag=f"acc{mt}", name=f"vacc{mt}")
                     for mt in range(2)]
            for hf in range(NHF):
                nc.sync.dma_start(out=xh[:, hf * H:(hf + 1) * H, :],
                                  in_=XH[:, hf * H:(hf + 1) * H, :])
                nc.scalar.dma_start(out=xl[:, hf * H:(hf + 1) * H, :],
                                    in_=XL[:, hf * H:(hf + 1) * H, :])
                wv = wp.tile([128, H, OS], f16, tag="w")
                wv_eng = nc.scalar if hf == 0 else nc.sync
                wv_eng.dma_start(out=wv[...], in_=WV[:, hf * H:(hf + 1) * H, :])
                for t in range(H):
                    kt = hf * H + t
                    for mt in range(2):
                        nc.tensor.matmul(out=vaccs[mt][:, :],
                                         lhsT=xh[:, kt, mt * 128:(mt + 1) * 128],
                                         rhs=wv[:, t, :],
                                         start=(kt == 0), stop=(kt == NKT - 1))
            for mt in range(2):
                v16 = sp.tile([128, OS], f16, tag=f"vsb{mt}")
                nc.scalar.copy(out=v16[:, :], in_=vaccs[mt][:, :])
                vsb[mt] = v16

            # ---- U tables: u[b,k] = v_b^T @ cw_k (overlaps q stream) ----
            uall = up.tile([128, B, K, 4, C], f16, tag="uall")
            for b in range(B):
                off = (b % 2) * C
                vt = vsb[b // 2][off:off + C, :]  # [64, 512] f16
                for k in range(K):
                    for m in range(4):
                        pt = pu.tile([128, C], f32, tag="pu", bufs=2)
                        nc.tensor.matmul(out=pt[:, :],
                                         lhsT=vt[:, m * 128:(m + 1) * 128],
                                         rhs=cw[off:off + C, k * C:(k + 1) * C],
                                         start=True, stop=True)
                        nc.scalar.copy(out=uall[:, b, k, m, :], in_=pt[:, :])
            nc.gpsimd.dma_start(out=UO[...], in_=uall[...])

            # ---- Q then K: split-3 fp16 matmuls ----
            qsb, ksb = {}, {}
            for which, WHt, WLt, store in (("q", WQH, WQL, qsb),
                                           ("k", WKH, WKL, ksb)):
                accs = [pp.tile([128, OS], f32, tag=f"acc{mt}",
                                name=f"acc_{which}{mt}") for mt in range(2)]
                for hf in range(NHF):
                    wh = wp.tile([128, H, OS], f16, tag="w")
                    wl = wp.tile([128, H, OS], f16, tag="w")
                    nc.sync.dma_start(out=wh[...], in_=WHt[:, hf * H:(hf + 1) * H, :])
                    nc.scalar.dma_start(out=wl[...], in_=WLt[:, hf * H:(hf + 1) * H, :])
                    last_hf = hf == NHF - 1
                    # mt-major on the last chunk: acc[mt] closes as soon as its
                    # own kt stream ends, so the fp16 split overlaps the rest
                    mt_t = ([(mt, t) for mt in range(2) for t in range(H)]
                            if last_hf else
                            [(mt, t) for t in range(H) for mt in range(2)])
                    for mt, t in mt_t:
                        kt = hf * H + t
                        first = (kt == 0)
                        last = (kt == NKT - 1)
                        lh = xh[:, kt, mt * 128:(mt + 1) * 128]
                        ll = xl[:, kt, mt * 128:(mt + 1) * 128]
                        nc.tensor.matmul(out=accs[mt][:, :], lhsT=lh,
                                         rhs=wh[:, t, :], start=first, stop=False)
                        nc.tensor.matmul(out=accs[mt][:, :], lhsT=lh,
                                         rhs=wl[:, t, :], start=False, stop=False)
                        nc.tensor.matmul(out=accs[mt][:, :], lhsT=ll,
                                         rhs=wh[:, t, :], start=False, stop=last)
                        if last_hf and t == H - 1:
                            res = sp.tile([128, OS], f32, tag=f"{which}sb{mt}",
                                          name=f"{which}sb{mt}")
                            nc.scalar.copy(out=res[:, :], in_=accs[mt][:, :])
                            store[mt] = res
                if which == "q":
                    # qn = q / ||q||_col + hi/lo split (overlaps k stream)
                    for mt in range(2):
                        qn = sp.tile([128, OS], f32, tag=f"qn{mt}", name=f"qn{mt}")
                        for half in range(2):
                            off = half * C
                            q_b = qsb[mt][off:off + C, :]
                            sq = sp.tile([128, OS], f32, tag="sq")
                            nc.scalar.square(out=sq[off:off + C, :], in_=q_b)
                            pn = pu.tile([1, OS], f32, tag="pn")
                            nc.tensor.matmul(out=pn[:, :], lhsT=ones_r[off:off + C, :],
                                             rhs=sq[off:off + C, :], start=True, stop=True)
                            nrm = sp.tile([1, OS], f32, tag="nrm")
                            nc.scalar.sqrt(out=nrm[:, :], in_=pn[:, :])
                            rcp = sp.tile([1, OS], f32, tag="rcp")
                            nc.vector.reciprocal(out=rcp[:, :], in_=nrm[:, :])
                            pb = pu.tile([128, OS], f32, tag="pb")
                            nc.tensor.matmul(out=pb[off:off + C, :], lhsT=ones_b[:, :],
                                             rhs=rcp[:, :], start=True, stop=True)
                            bc = sp.tile([128, OS], f32, tag="bc")
                            nc.scalar.copy(out=bc[off:off + C, :], in_=pb[off:off + C, :])
                            nc.vector.tensor_mul(out=qn[off:off + C, :], in0=q_b,
                                                 in1=bc[off:off + C, :])
                        qh16 = sp.tile([128, OS], f16, tag=f"qh16_{mt}", name=f"qh16_{mt}")
                        nc.scalar.copy(out=qh16[:, :], in_=qn[:, :])
                        qh32 = sp.tile([128, OS], f32, tag="qh32")
                        nc.scalar.copy(out=qh32[:, :], in_=qh16[:, :])
                        ql16 = sp.tile([128, OS], f16, tag=f"ql16_{mt}", name=f"ql16_{mt}")
                        nc.vector.tensor_sub(out=ql16[:, :], in0=qn[:, :], in1=qh32[:, :])
                        nc.sync.dma_start(out=QNH[mt], in_=qh16[:, :])
                        nc.sync.dma_start(out=QNL[mt], in_=ql16[:, :])

            # ---- k hi/lo split + store ----
            for mt in range(2):
                kh16 = sp.tile([128, OS], f16, tag=f"kh16_{mt}", name=f"kh16_{mt}")
                nc.scalar.copy(out=kh16[:, :], in_=ksb[mt][:, :])
                kh32 = sp.tile([128, OS], f32, tag="kh32")
                nc.scalar.copy(out=kh32[:, :], in_=kh16[:, :])
                kl16 = sp.tile([128, OS], f16, tag=f"kl16_{mt}", name=f"kl16_{mt}")
                nc.vector.tensor_sub(out=kl16[:, :], in0=ksb[mt][:, :], in1=kh32[:, :])
                nc.sync.dma_start(out=KH[mt], in_=kh16[:, :])
                nc.sync.dma_start(out=KL[mt], in_=kl16[:, :])
    nc.compile()
    return nc


def _build_l2():
    """Phase 2, core j: rows t in [j*512,(j+1)*512) for all batches.
    sim via packed split-3 (2 matmuls per 512-chunk), exact top-4 via
    max/max_index on fp32, gather-sum from combined u table, partial out."""
    nc = bacc.Bacc("TRN2", target_bir_lowering=False, debug=False, num_devices=NCORES)
    QHD = nc.dram_tensor("qhd", [B, 128, T], f16, kind="ExternalInput").ap()
    QLP = nc.dram_tensor("qlp", [2, 128, T], f16, kind="ExternalInput").ap()
    KJP = nc.dram_tensor("kjp", [B, 128, OS], f16, kind="ExternalInput").ap()
    WOT = nc.dram_tensor("wot", [128, 4, T], f16, kind="ExternalInput").ap()
    UT = [nc.dram_tensor(f"ut{b}", [K * T, C], f16, kind="ExternalInput").ap()
          for b in range(B)]
    OUT = nc.dram_tensor("out_o", [2, 128, T], f32, kind="ExternalOutput").ap()

    NCH = T // 512  # 8 s-chunks

    with tile.TileContext(nc) as tc:
        with tc.tile_pool(name="qp", bufs=1) as qp, \
             tc.tile_pool(name="wp", bufs=1) as wp, \
             tc.tile_pool(name="sp", bufs=3) as sp, \
             tc.tile_pool(name="simp", bufs=2) as simp, \
             tc.tile_pool(name="yp", bufs=1) as yp, \
             tc.tile_pool(name="pp", bufs=2, space="PSUM") as pp, \
             tc.tile_pool(name="po", bufs=2, space="PSUM") as po:
            kjp, qhd, qlp = {}, {}, {}
            kt0 = qp.tile([128, OS], f16, tag="kjp0", name="kjp0")
            nc.sync.dma_start(out=kt0[...], in_=KJP[0])
            kjp[0] = kt0
            # b=0 q tiles stream in quarters so block (0,0) starts early
            qt0 = qp.tile([128, T], f16, tag="qhd0", name="qhd0")
            ql0 = qp.tile([128, T], f16, tag="qlp0", name="qlp0")
            for c4 in range(4):
                nc.sync.dma_start(out=qt0[:, c4 * 1024:(c4 + 1) * 1024],
                                  in_=QHD[0][:, c4 * 1024:(c4 + 1) * 1024])
                nc.sync.dma_start(out=ql0[:, c4 * 1024:(c4 + 1) * 1024],
                                  in_=QLP[0][:, c4 * 1024:(c4 + 1) * 1024])
            qhd[0] = qt0
            qlp[0] = ql0
            for b in range(1, B):
                kt_ = qp.tile([128, OS], f16, tag=f"kjp{b}", name=f"kjp{b}")
                nc.sync.dma_start(out=kt_[...], in_=KJP[b])
                kjp[b] = kt_
            for b in range(1, B):
                qt = qp.tile([128, T], f16, tag=f"qhd{b}", name=f"qhd{b}")
                nc.sync.dma_start(out=qt[...], in_=QHD[b])
                qhd[b] = qt
            qt = qp.tile([128, T], f16, tag="qlp1", name="qlp1")
            nc.scalar.dma_start(out=qt[...], in_=QLP[1])
            qlp[1] = qt
            wot = wp.tile([128, 4, T], f16, tag="wot")
            for kt in range(4):
                nc.sync.dma_start(out=wot[:, kt, :], in_=WOT[:, kt, :])

            ytp = {}
            for pair in range(2):
                for kt in range(4):
                    ytp[(pair, kt)] = yp.tile([128, 128], f16, tag=f"yt{pair}{kt}", name=f"ytp{pair}{kt}")

            def do_block(b, i):
                """sim+topk+gather for token block i (128 rows) of batch b."""
                loff = (b % 2) * C  # partition offset of kh within kjp[b]
                lhs_full = kjp[b][:, i * 128:(i + 1) * 128]
                lhs_h = kjp[b][loff:loff + C, i * 128:(i + 1) * 128]
                sim = simp.tile([128, T], f32, tag="sim", bufs=3)
                for ch2 in range(NCH // 2):
                    ps = pp.tile([128, 1024], f32, tag="ps")
                    for half in range(2):
                        ch = ch2 * 2 + half
                        po_s = ps[:, half * 512:(half + 1) * 512]
                        nc.tensor.matmul(out=po_s, lhsT=lhs_full,
                                         rhs=qhd[b][:, ch * 512:(ch + 1) * 512],
                                         start=True, stop=False)
                        nc.tensor.matmul(out=po_s, lhsT=lhs_h,
                                         rhs=qlp[b // 2][loff:loff + C,
                                                         ch * 512:(ch + 1) * 512],
                                         start=False, stop=True)
                    nc.scalar.copy(out=sim[:, ch2 * 1024:(ch2 + 1) * 1024],
                                   in_=ps[:, :])
                m8 = sp.tile([128, 8], f32, tag="m8")
                i8 = sp.tile([128, 8], u32, tag="i8")
                nc.vector.max(out=m8[:, :], in_=sim[:, :])
                nc.vector.max_index(out=i8[:, :], in_max=m8[:, :], in_values=sim[:, :])
                gth = sp.tile([128, K, C], f16, tag="gth")
                for k in range(K):
                    nc.gpsimd.indirect_dma_start(
                        out=gth[:, k, :], out_offset=None,
                        in_=UT[b][:, :],
                        in_offset=bass.IndirectOffsetOnAxis(ap=i8[:, k:k + 1], axis=0),
                        element_offset=k * T * C)
                t0 = sp.tile([128, C], f16, tag="t0")
                t1 = sp.tile([128, C], f16, tag="t1")
                nc.gpsimd.tensor_add(out=t0[:, :], in0=gth[:, 0, :], in1=gth[:, 1, :])
                nc.gpsimd.tensor_add(out=t1[:, :], in0=gth[:, 2, :], in1=gth[:, 3, :])
                dst = ytp[(b // 2, i)][:, (b % 2) * C:(b % 2) * C + C]
                nc.gpsimd.tensor_add(out=dst, in0=t0[:, :], in1=t1[:, :])

            out_ps = {}

            def out_prefill(pair):
                # accumulate kt=0..2 into 4 open psum groups (4 banks) before
                # the pair's last token block lands
                for ch2 in range(NCH // 2):
                    ps = po.tile([128, 512], f32, tag="po", bufs=4,
                                 name=f"po{pair}{ch2}")
                    out_ps[(pair, ch2)] = ps
                    for kt in range(3):
                        nc.tensor.matmul(out=ps[:, :], lhsT=ytp[(pair, kt)][:, :],
                                         rhs=wot[:, kt, ch2 * 512:(ch2 + 1) * 512],
                                         start=(kt == 0), stop=False)

            def do_out(pair):
                ob = simp.tile([128, T], f32, tag="ob")
                for ch2 in range(NCH // 2):
                    ps = out_ps[(pair, ch2)]
                    nc.tensor.matmul(out=ps[:, :], lhsT=ytp[(pair, 3)][:, :],
                                     rhs=wot[:, 3, ch2 * 512:(ch2 + 1) * 512],
                                     start=False, stop=True)
                    nc.scalar.copy(out=ob[:, ch2 * 512:(ch2 + 1) * 512], in_=ps[:, :])
                    eng = nc.sync if ch2 % 2 == 0 else nc.gpsimd
                    eng.dma_start(out=OUT[pair][:, ch2 * 512:(ch2 + 1) * 512],
                                  in_=ob[:, ch2 * 512:(ch2 + 1) * 512])
                for ch2 in range(NCH // 2, NCH):
                    ps = po.tile([128, 512], f32, tag="po", bufs=4,
                                 name=f"po{pair}{ch2}")
                    for kt in range(4):
                        nc.tensor.matmul(out=ps[:, :], lhsT=ytp[(pair, kt)][:, :],
                                         rhs=wot[:, kt, ch2 * 512:(ch2 + 1) * 512],
                                         start=(kt == 0), stop=(kt == 3))
                    nc.scalar.copy(out=ob[:, ch2 * 512:(ch2 + 1) * 512], in_=ps[:, :])
                    eng = nc.sync if ch2 % 2 == 0 else nc.gpsimd
                    eng.dma_start(out=OUT[pair][:, ch2 * 512:(ch2 + 1) * 512],
                                  in_=ob[:, ch2 * 512:(ch2 + 1) * 512])

            for b in range(2):
                for i in range(4):
                    if b == 1 and i == 3:
                        out_prefill(0)
                    do_block(b, i)
            do_out(0)
            for b in range(2, 4):
                for i in range(4):
                    if b == 3 and i == 3:
                        out_prefill(1)
                    do_block(b, i)
            do_out(1)
    nc.compile()
    return nc


def _split16(a):
    h = a.astype(np.float16)
    l = (a.astype(np.float32) - h.astype(np.float32)).astype(np.float16)
    return h, l


def _sw(a):
    # [T, W] -> [128, T//128, W] with [p, kt, w] = a[kt*128+p, w]
    return np.ascontiguousarray(a.reshape(T // 128, 128, -1).transpose(1, 0, 2))


def kernel(x, Wq, Wk, Wv, Wo, conv_w, conv_b):
    x = np.asarray(x, np.float32)
    Wq = np.asarray(Wq, np.float32); Wk = np.asarray(Wk, np.float32)
    Wv = np.asarray(Wv, np.float32); Wo = np.asarray(Wo, np.float32)
    conv_w = np.asarray(conv_w, np.float32); conv_b = np.asarray(conv_b, np.float32)

    if "l1" not in _cache:
        _cache["l1"] = _build_l1()
    if "l2" not in _cache:
        _cache["l2"] = _build_l2()

    xT = np.ascontiguousarray(x.transpose(2, 0, 1).reshape(T, BC))  # [t, b*64+c]
    xh, xl = _split16(xT)
    xh, xl = _sw(xh), _sw(xl)
    WqT, WkT = Wq.T, Wk.T
    WvT16 = np.ascontiguousarray(Wv.T).astype(np.float16)
    cw1 = np.ascontiguousarray(conv_w.transpose(1, 2, 0).reshape(C, K * C)).astype(np.float16)
    cw = np.concatenate([cw1, cw1], axis=0)

    in_maps = []
    for j in range(NCORES):
        sl = slice(j * OS, (j + 1) * OS)
        wqh, wql = _split16(np.ascontiguousarray(WqT[:, sl]))
        wkh, wkl = _split16(np.ascontiguousarray(WkT[:, sl]))
        in_maps.append({"xh": xh, "xl": xl,
                        "wqh": _sw(wqh), "wql": _sw(wql),
                        "wkh": _sw(wkh), "wkl": _sw(wkl),
                        "wv": _sw(np.ascontiguousarray(WvT16[:, sl])), "cw": cw})
    r1 = bass_utils.run_bass_kernel_spmd(_cache["l1"], in_maps, core_ids=list(range(NCORES)))

    # host: assemble full qn hi/lo [BC, T], k hi/lo, u tables
    qh_full = np.concatenate([r1.results[j]["qnh_o"].reshape(BC, OS)
                              for j in range(NCORES)], axis=1)  # [256, T]
    ql_full = np.concatenate([r1.results[j]["qnl_o"].reshape(BC, OS)
                              for j in range(NCORES)], axis=1)
    ut = {}
    for b in range(B):
        # u_o[p, b, k, m, c] -> table [K*T, C] rows k*T + (j*512 + m*128 + p)
        tabs = []
        for k in range(K):
            cols = [r1.results[j]["u_o"][:, b, k].transpose(1, 0, 2).reshape(OS, C)
                    for j in range(NCORES)]
            tabs.append(np.concatenate(cols, axis=0))  # [T, C]
        ut[b] = np.ascontiguousarray(np.concatenate(tabs, axis=0))  # [K*T, C]

    # per-batch duplicated qh and packed ql
    qhd = np.empty((B, 128, T), np.float16)
    qlp = np.empty((2, 128, T), np.float16)
    for b in range(B):
        qh_b = qh_full[b * C:(b + 1) * C]  # [64, T]
        qhd[b, :C] = qh_b
        qhd[b, C:] = qh_b
        qlp[b // 2, (b % 2) * C:(b % 2) * C + C] = ql_full[b * C:(b + 1) * C]

    wotT = np.ascontiguousarray(Wo.T).astype(np.float16)  # [T, T] rows t

    in_maps2 = []
    for j in range(NCORES):
        sl = slice(j * OS, (j + 1) * OS)
        khj = r1.results[j]["kh_o"].reshape(BC, OS)
        klj = r1.results[j]["kl_o"].reshape(BC, OS)
        kjp = np.empty((B, 128, OS), np.float16)
        for b in range(B):
            kh_b = khj[b * C:(b + 1) * C]
            kl_b = klj[b * C:(b + 1) * C]
            if b % 2 == 0:
                kjp[b, :C], kjp[b, C:] = kh_b, kl_b
            else:
                kjp[b, :C], kjp[b, C:] = kl_b, kh_b
        wotj = np.ascontiguousarray(
            wotT[sl, :].reshape(4, 128, T).transpose(1, 0, 2))
        m = {"qhd": qhd, "qlp": qlp, "kjp": kjp, "wot": wotj}
        for b in range(B):
            m[f"ut{b}"] = ut[b]
        in_maps2.append(m)
    r2 = bass_utils.run_bass_kernel_spmd(_cache["l2"], in_maps2, core_ids=list(range(NCORES)))

    out = np.zeros((B, C, T), np.float32)
    for j in range(NCORES):
        oo = r2.results[j]["out_o"]  # [2, 128, T]
        for b in range(B):
            out[b] += oo[b // 2, (b % 2) * C:(b % 2) * C + C, :]
    out += (conv_b[:, None] * Wo.sum(axis=1)[None, :])[None, :, :]
    return out
